# revision 1
# baseline (speedup 1.0000x reference)
"""Trainium2 Bass kernel for nn_BottomLevelDecoderRNN.

Structure exploited: the recurrent state is reset at every bar boundary
(t % 16 == 0) and `notes` is teacher-forced from `target`, so the 16 bars
of 16 steps each are fully independent. We therefore run a 16-step loop
with (bar, batch) vmapped into a 256-wide column dimension per core
(batch is sharded 8 ways across cores; 16 bars x 16 batch = 256 columns).

All on-device tensors are kept transposed: [feature -> partitions (folded
128x2), (bar,batch) -> free dim], so the LSTM chain needs no transposes.
Matmul operands are fp16 (1 cycle/row on the PE), accumulation fp32.

Host precomputes exactly (fp32): h_init per bar, the c_t input
contribution xc1 (constant within a bar, shared by all 9 lstm1-type cell
evaluations per step, biases folded in), and all note embeddings.
"""

import numpy as np

import concourse.bacc as bacc
import concourse.mybir as mybir
import concourse.tile as tile
from concourse.bass import MemorySpace
from concourse.bass_utils import run_bass_kernel_spmd
from concourse.masks import make_identity

B, Dd, Hh, Vv = 128, 256, 256, 130
NB = 16          # bars
BL = B // 8      # batch per core
R = NB * BL      # columns per core = 256
S = 16           # steps per bar
NCORES = 8
F16 = mybir.dt.float16
F32 = mybir.dt.float32
AF = mybir.ActivationFunctionType

last_result = None  # BassKernelResults of the most recent run (for profiling)
_prog_cache = {}
ORDER = 8  # emission-order variant (scheduler priority tuning)
REPS = 1   # >1 repeats the whole 16-step body (timing amplification only)


def _foldT(M):
    """M [X cols, Rd rows] -> tile [128, (Rd/128)*X]; tile[p, q*X+x] = M[x, q*128+p]."""
    X, Rd = M.shape
    q = Rd // 128
    return np.ascontiguousarray(M.reshape(X, q, 128).transpose(2, 1, 0).reshape(128, q * X))


def _wT(W, in_dim):
    """W [G, in_dim] -> [in_dim//128, 128, G] chunks of W.T"""
    G = W.shape[0]
    return np.ascontiguousarray(W.reshape(G, in_dim // 128, 128).transpose(1, 2, 0))


def _build_program(use_ctx_bias):
    nc = bacc.Bacc(None, target_bir_lowering=False)

    # ---- DRAM I/O ----
    d_w1n = nc.dram_tensor("w1n", [3, 2, 128, 1024], F16, kind="ExternalInput")
    d_w1h = nc.dram_tensor("w1h", [3, 2, 128, 1024], F16, kind="ExternalInput")
    d_wci = nc.dram_tensor("wci", [6, 128, 1024], F16, kind="ExternalInput")
    d_wch = nc.dram_tensor("wch", [2, 128, 1024], F16, kind="ExternalInput")
    d_wo = nc.dram_tensor("wo", [3, 2, 128, 130], F16, kind="ExternalInput")
    d_xc1 = nc.dram_tensor("xc1", [3, 128, 2048], F16, kind="ExternalInput")
    d_hinit = nc.dram_tensor("hinit", [128, 512], F16, kind="ExternalInput")
    d_xa0 = nc.dram_tensor("xa0", [3, 128, 2048], F16, kind="ExternalInput")
    d_xb = nc.dram_tensor("xb", [S, 3, 128, 2048], F16, kind="ExternalInput")
    d_boutA = nc.dram_tensor("boutA", [3, 128, 1], F32, kind="ExternalInput")
    d_boutB = nc.dram_tensor("boutB", [3, 2, 1], F32, kind="ExternalInput")
    if use_ctx_bias:
        d_bcb = nc.dram_tensor("bcb", [128, 2048], F16, kind="ExternalInput")
    d_out = nc.dram_tensor("out", [S, 3, 130, R], F32, kind="ExternalOutput")

    from contextlib import ExitStack
    with TileCtx(nc) as tc, ExitStack() as es:
        const = es.enter_context(tc.tile_pool(name="const", bufs=1))
        psum_ctx = es.enter_context(tc.tile_pool(name="psum_ctx", bufs=2, space=MemorySpace.PSUM))
        psum = es.enter_context(tc.tile_pool(name="psum", bufs=6, space=MemorySpace.PSUM))
        scr = es.enter_context(tc.tile_pool(name="scr", bufs=3))
        scr2 = es.enter_context(tc.tile_pool(name="scr2", bufs=2))
        stg = es.enter_context(tc.tile_pool(name="stg", bufs=3))
        npool = es.enter_context(tc.tile_pool(name="npool", bufs=3))
        hpool = es.enter_context(tc.tile_pool(name="hpool", bufs=3))
        cpool = es.enter_context(tc.tile_pool(name="cpool", bufs=2))

        def cload(name, dram_ap, shape, dtype):
            t = const.tile(shape, dtype, tag=name)
            nc.sync.dma_start(t[:], dram_ap)
            return t

        # consts needed by the first cells load first (vmap: w1h/xa0/hinit,
        # then ctx: wci/wch) so the PE starts before the full preload finishes
        hinit = cload("hinit", d_hinit[:], [128, 512], F16)
        xa0 = [cload(f"xa0_{i}", d_xa0[i], [128, 2048], F16) for i in range(3)]
        w1h = [[cload(f"w1h_{i}_{k}", d_w1h[i, k], [128, 1024], F16) for k in range(2)]
               for i in range(3)]
        wci = [cload(f"wci_{k}", d_wci[k], [128, 1024], F16) for k in range(6)]
        wch = [cload(f"wch_{k}", d_wch[k], [128, 1024], F16) for k in range(2)]
        w1n = [[cload(f"w1n_{i}_{k}", d_w1n[i, k], [128, 1024], F16) for k in range(2)]
               for i in range(3)]
        xc1 = [cload(f"xc1_{i}", d_xc1[i], [128, 2048], F16) for i in range(3)]
        wo = [[cload(f"wo_{i}_{k}", d_wo[i, k], [128, 130], F16) for k in range(2)]
              for i in range(3)]
        boutA = [cload(f"boutA_{i}", d_boutA[i], [128, 1], F32) for i in range(3)]
        boutB = [cload(f"boutB_{i}", d_boutB[i], [2, 1], F32) for i in range(3)]
        bcb = cload("bcb", d_bcb[:], [128, 2048], F16) if use_ctx_bias else None

        ident = const.tile([128, 128], F16, tag="ident")
        make_identity(nc, ident[:])
        zeros = const.tile([128, 512], F32, tag="zeros")
        nc.gpsimd.memset(zeros[:], 0.0)

        def lstm_cell(ih_pairs, hh_w, h_tile, xadd, c_tile, htag, ctag,
                      gpool=None):
            """ih_pairs: list of (w_tile, rhs_ap[128,256]) for the input part.
            hh_w: [2 tiles] recurrent weights; h_tile: current h [128,512] f16.
            xadd: [128,2048] f16 precomputed additive input-term (or None).
            Gates land in four single-bank PSUM tiles (i, f, g, o), each
            holding one folded [256,256] gate. Returns (h_new f16, c_new f32)."""
            pairs = list(ih_pairs) + [(hh_w[k], h_tile[:, k * R:(k + 1) * R])
                                      for k in range(2)]
            gp = gpool if gpool is not None else psum
            gt = []
            for gi in range(4):  # i, f, g, o
                pt = gp.tile([128, 512], F32, tag="g", name=f"gt{gi}")
                gt.append(pt)
                if xadd is not None:
                    nc.tensor.matmul(
                        pt[:], ident[:], xadd[:, gi * 512:(gi + 1) * 512],
                        start=True, stop=False, skip_group_check=True)
                for q in range(2):  # lo/hi fold chunk
                    m = gi * 2 + q
                    outap = pt[:, q * R:(q + 1) * R]
                    for j, (wt, rhs) in enumerate(pairs):
                        nc.tensor.matmul(
                            outap, wt[:, m * 128:(m + 1) * 128], rhs,
                            start=(xadd is None and j == 0),
                            stop=(j == len(pairs) - 1),
                            skip_group_check=True)
            a_i = scr.tile([128, 512], F32, tag="a_i")
            nc.scalar.activation(a_i[:], gt[0][:], AF.Sigmoid)
            a_f = scr.tile([128, 512], F32, tag="a_f")
            nc.scalar.activation(a_f[:], gt[1][:], AF.Sigmoid)
            a_g = scr.tile([128, 512], F32, tag="a_g")
            nc.scalar.activation(a_g[:], gt[2][:], AF.Tanh)
            a_o = scr.tile([128, 512], F32, tag="a_o")
            nc.scalar.activation(a_o[:], gt[3][:], AF.Sigmoid)
            m1 = scr2.tile([128, 512], F32, tag="m1")
            nc.vector.tensor_mul(m1[:], a_i[:], a_g[:])
            c_new = cpool.tile([128, 512], F32, tag=ctag)
            nc.vector.tensor_mul(c_new[:], a_f[:], c_tile[:])
            nc.vector.tensor_add(c_new[:], c_new[:], m1[:])
            tc2 = scr2.tile([128, 512], F32, tag="tc2")
            nc.scalar.activation(tc2[:], c_new[:], AF.Tanh)
            h_new = hpool.tile([128, 512], F16, tag=htag)
            nc.vector.tensor_mul(h_new[:], a_o[:], tc2[:])
            return h_new, c_new

        h1 = [hinit, hinit, hinit]
        h2 = [hinit, hinit, hinit]
        hc = hinit
        c1 = [zeros, zeros, zeros]
        c2 = [zeros, zeros, zeros]
        cc = zeros

        xb_prev = None
        pending_outs = []
        for rep_s in range(S * REPS):
            rep, s = divmod(rep_s, S)
            # stream this step's combined additive input term (teacher-forced
            # note contribution + xc1 + biases), prefetched by Tile
            xb_cur = []
            for i in range(3):
                t = npool.tile([128, 2048], F16, tag=f"xb_{i}")
                nc.sync.dma_start(t[:], d_xb[s, i])
                xb_cur.append(t)
            xa = xa0 if rep_s == 0 else xb_prev
            # vmap lstm1 update for the 3 streams (input term fully additive)
            for i in range(3):
                h1[i], c1[i] = lstm_cell(
                    [], w1h[i], h1[i], xa[i], c1[i], f"h1_{i}", f"c1_{i}")
            h1v = list(h1)  # post-vmap values (read by ctx and out)

            def again(i):
                h1[i], c1[i] = lstm_cell(
                    [], w1h[i], h1[i], xb_cur[i], c1[i], f"h1_{i}", f"c1_{i}")

            def ctx(i):
                # ctx lstm reads h1 = [again(j) for j < i, vmap(j) for j >= i]
                hin = [h1[j] if j < i else h1v[j] for j in range(3)]
                return lstm_cell(
                    [(wci[j * 2 + k], hin[j][:, k * R:(k + 1) * R])
                     for j in range(3) for k in range(2)],
                    wch, hc, bcb, cc, "hc", "cc", gpool=psum_ctx)

            def lstm2(i):
                h2[i], c2[i] = lstm_cell(
                    [(w1n[i][k], hcs[i][:, k * R:(k + 1) * R]) for k in range(2)],
                    w1h[i], h2[i], xc1[i], c2[i], f"h2_{i}", f"c2_{i}")

            def out_proj(i):
                out_proj_c(i, h1v[i], h2[i], s)

            def out_proj_c(i, h1v_i, h2_i, s):
                # out projection: (h1v[i] + h2[i]) @ Wout[i].T + bout[i]
                hsum = scr.tile([128, 512], F16, tag="hsum")
                nc.vector.tensor_add(hsum[:], h1v_i[:], h2_i[:])
                tout = psum.tile([128, 512], F32, tag="g")
                for k in range(2):
                    nc.tensor.matmul(tout[:, 0:R], wo[i][k][:, 0:128],
                                     hsum[:, k * R:(k + 1) * R],
                                     start=(k == 0), stop=(k == 1),
                                     skip_group_check=True)
                for k in range(2):
                    nc.tensor.matmul(tout[0:2, R:2 * R], wo[i][k][:, 128:130],
                                     hsum[:, k * R:(k + 1) * R],
                                     start=(k == 0), stop=(k == 1),
                                     skip_group_check=True)
                stage = stg.tile([128, 512], F32, tag="stage")
                nc.vector.tensor_scalar_add(stage[:, 0:R], tout[:, 0:R],
                                            boutA[i][:])
                nc.vector.tensor_scalar_add(stage[0:2, R:2 * R],
                                            tout[0:2, R:2 * R], boutB[i][:])
                nc.sync.dma_start(d_out[s, i, 0:128, :], stage[:, 0:R])
                nc.sync.dma_start(d_out[s, i, 128:130, :], stage[0:2, R:2 * R])

            def lstm2_and_out(i):
                lstm2(i)
                out_proj(i)

            # emission order = scheduler priority: keep the serial ctx chain
            # hot; "again" and lstm2/out work fills PE gaps.
            hcs = [None, None, None]
            if ORDER == 0:
                hc, cc = ctx(0); hcs[0] = hc
                again(0)
                hc, cc = ctx(1); hcs[1] = hc
                again(1)
                hc, cc = ctx(2); hcs[2] = hc
                again(2)
                for i in range(3):
                    lstm2_and_out(i)
            elif ORDER == 1:
                hc, cc = ctx(0); hcs[0] = hc
                again(0)
                hc, cc = ctx(1); hcs[1] = hc
                again(1)
                lstm2_and_out(0)
                hc, cc = ctx(2); hcs[2] = hc
                again(2)
                lstm2_and_out(1)
                lstm2_and_out(2)
            elif ORDER == 2:
                again(0)
                hc, cc = ctx(0); hcs[0] = hc
                again(1)
                hc, cc = ctx(1); hcs[1] = hc
                again(2)
                hc, cc = ctx(2); hcs[2] = hc
                for i in range(3):
                    lstm2_and_out(i)
            elif ORDER == 4:
                hc, cc = ctx(0); hcs[0] = hc
                again(0)
                lstm2_and_out(0)
                hc, cc = ctx(1); hcs[1] = hc
                again(1)
                lstm2_and_out(1)
                hc, cc = ctx(2); hcs[2] = hc
                again(2)
                lstm2_and_out(2)
            elif ORDER == 5:
                hc, cc = ctx(0); hcs[0] = hc
                again(0)
                hc, cc = ctx(1); hcs[1] = hc
                lstm2_and_out(0)
                again(1)
                hc, cc = ctx(2); hcs[2] = hc
                lstm2_and_out(1)
                again(2)
                lstm2_and_out(2)
            elif ORDER == 6:
                hc, cc = ctx(0); hcs[0] = hc
                lstm2_and_out(0)
                again(0)
                hc, cc = ctx(1); hcs[1] = hc
                lstm2_and_out(1)
                again(1)
                hc, cc = ctx(2); hcs[2] = hc
                lstm2_and_out(2)
                again(2)
            elif ORDER == 7:  # like 4, but output projections deferred
                hc, cc = ctx(0); hcs[0] = hc
                again(0)
                lstm2(0)
                hc, cc = ctx(1); hcs[1] = hc
                again(1)
                lstm2(1)
                hc, cc = ctx(2); hcs[2] = hc
                again(2)
                lstm2(2)
                for i in range(3):
                    out_proj(i)
            else:  # ORDER == 8 variant: outs deferred into the next step
                hc, cc = ctx(0); hcs[0] = hc
                again(0)
                hc, cc = ctx(1); hcs[1] = hc
                again(1)
                hc, cc = ctx(2); hcs[2] = hc
                again(2)
                for i in range(3):
                    lstm2(i)
                for fn in pending_outs:
                    fn()
                pending_outs = [
                    (lambda i=i, h1v_=h1v, h2_=h2[i], s_=s:
                     out_proj_c(i, h1v_[i], h2_, s_)) for i in range(3)]
            xb_prev = xb_cur
        for fn in pending_outs:
            fn()

    nc.compile()
    return nc


def TileCtx(nc):
    return tile.TileContext(nc)


def kernel(c, target, length, W_hid, b_hid, W1_ih, W1_hh, b1_ih, b1_hh,
           Wc_ih, Wc_hh, bc_ih, bc_hh, emb, Wout, bout):
    global last_result
    c = np.asarray(c, np.float32)
    tgt = np.asarray(target).astype(np.int64)
    W_hid = np.asarray(W_hid, np.float32)
    b_hid = np.asarray(b_hid, np.float32)
    W1_ih = np.asarray(W1_ih, np.float32)
    W1_hh = np.asarray(W1_hh, np.float32)
    b1 = np.asarray(b1_ih, np.float32) + np.asarray(b1_hh, np.float32)
    Wc_ih = np.asarray(Wc_ih, np.float32)
    Wc_hh = np.asarray(Wc_hh, np.float32)
    bc = np.asarray(bc_ih, np.float32) + np.asarray(bc_hh, np.float32)
    emb = np.asarray(emb, np.float32)
    Wout = np.asarray(Wout, np.float32)
    bout = np.asarray(bout, np.float32)
    L = int(length)
    assert L == NB * S and c.shape == (B, NB + 1, Dd)

    f16 = np.float16
    use_ctx_bias = bool(np.any(bc != 0.0))

    # ---- replicated weight prep ----
    w1n = np.stack([_wT(W1_ih[i][:, :Dd], Dd) for i in range(3)]).astype(f16)
    w1h = np.stack([_wT(W1_hh[i], Hh) for i in range(3)]).astype(f16)
    wci = _wT(Wc_ih, 3 * Hh).astype(f16)
    wch = _wT(Wc_hh, Hh).astype(f16)
    wo = np.stack([_wT(Wout[i], Hh) for i in range(3)]).astype(f16)
    boutA = np.ascontiguousarray(bout[:, :128, None])
    boutB = np.ascontiguousarray(bout[:, 128:130, None])
    bcb = _foldT(np.broadcast_to(bc[None, :], (R, 4 * Hh))).astype(f16)

    # full-batch fp32 precomputes
    h_init_full = np.tanh(np.einsum('bnd,hd->bnh', c[:, :NB], W_hid[:Hh]) + b_hid[:Hh])
    # note contribution per vocab entry: NEt[i] = emb[i] @ W1n[i].T  [V, 4H]
    NEt = np.stack([emb[i] @ W1_ih[i][:, :Dd].T for i in range(3)])
    in_maps = []
    for r in range(NCORES):
        cs = c[r * BL:(r + 1) * BL]           # [BL, 17, D]
        CT = cs[:, 1:NB + 1].transpose(1, 0, 2).reshape(R, Dd)      # x=(bar,bl)
        HI = h_init_full[r * BL:(r + 1) * BL].transpose(1, 0, 2).reshape(R, Hh)
        xc1f = [CT @ W1_ih[i][:, Dd:].T + b1[i] for i in range(3)]  # [R, 4H]
        xc1 = np.stack([_foldT(x) for x in xc1f]).astype(f16)
        hinit = _foldT(HI).astype(f16)
        tg = tgt[:, r * BL:(r + 1) * BL]      # [3, BL, 256]
        # notes entering step 0: bar0 -> token 0; else target at bar*16-1
        tokA0 = np.empty((3, R), np.int64)
        for i in range(3):
            tokA0[i] = np.concatenate(
                [np.zeros(BL, np.int64)] +
                [tg[i, :, bar * S - 1] for bar in range(1, NB)])
        xa0 = np.stack([_foldT(NEt[i][tokA0[i]] + xc1f[i])
                        for i in range(3)]).astype(f16)
        # combined additive input term at step s (teacher forcing)
        tr = tg.reshape(3, BL, NB, S)         # [i, bl, bar, s]
        xbarr = np.empty((S, 3, 128, 2048), f16)
        for s in range(S):
            for i in range(3):
                toks = tr[i, :, :, s].T.reshape(R)   # x = bar*BL+bl
                xbarr[s, i] = _foldT(NEt[i][toks] + xc1f[i]).astype(f16)
        m = dict(w1n=w1n, w1h=w1h, wci=wci, wch=wch, wo=wo, xc1=xc1,
                 hinit=hinit, xa0=xa0, xb=xbarr, boutA=boutA, boutB=boutB)
        if use_ctx_bias:
            m["bcb"] = bcb
        in_maps.append(m)

    key = use_ctx_bias
    if key not in _prog_cache:
        _prog_cache[key] = _build_program(use_ctx_bias)
    nc = _prog_cache[key]

    last_result = run_bass_kernel_spmd(nc, in_maps, core_ids=list(range(NCORES)))

    out_full = np.empty((3, B, L, Vv), np.float32)
    for r in range(NCORES):
        A = last_result.results[r]["out"]          # [S, 3, 130, R]
        A = A.reshape(S, 3, Vv, NB, BL).transpose(1, 4, 3, 0, 2)  # [3, bl, bar, s, V]
        out_full[:, r * BL:(r + 1) * BL] = A.reshape(3, BL, L, Vv)
    return out_full



# revision 5
# speedup vs baseline: 1.0568x; 1.0568x over previous
"""Trainium2 Bass kernel for nn_BottomLevelDecoderRNN.

Structure exploited: the recurrent state is reset at every bar boundary
(t % 16 == 0) and `notes` is teacher-forced from `target`, so the 16 bars
of 16 steps each are fully independent. We run a 16-step loop with
(bar, batch) vmapped into a 256-wide column dimension per core (batch is
sharded 8 ways across cores; 16 bars x 16 batch = 256 columns).

v2 engine plan (cost-model driven):
- All recurrent gate matmuls are fp8-e4m3 DoubleRow (K=256 per MM, 2x PE
  rate and half the instructions of fp16).  The precomputed additive
  input term (note embedding + c_t part + biases) stays exact fp16 and is
  injected into PSUM via identity matmuls.  The output projection stays
  fp16 (logit precision).
- Gates are reordered host-side to [i, f, o, 2*g] so ONE merged Sigmoid
  over the whole 4-bank PSUM gate tile [128, 2048] serves all four gates:
  tanh(g) = 2*sigmoid(2g) - 1, with the 2x pre-scaled into the weights.
- tanh(c_new) is batched across the 3 independent cells of each phase
  (one [128,1536] op); ctx cells (serial chain) get individual tanh.
- DVE elementwise runs fp16 (2x mode); h state is written fp8 (for the
  DoubleRow rhs) and also fp16 where the output projection needs it.
- The sigmoid(f)*c multiplies of off-spine cells run on the idle GPSIMD.
"""

import numpy as np
import ml_dtypes

import concourse.bacc as bacc
import concourse.mybir as mybir
import concourse.tile as tile
from concourse.bass import MemorySpace
from concourse.bass_utils import run_bass_kernel_spmd
from concourse.masks import make_identity

B, Dd, Hh, Vv = 128, 256, 256, 130
NB = 16          # bars
BL = B // 8      # batch per core
R = NB * BL      # columns per core = 256
S = 16           # steps per bar
NCORES = 8
F16 = mybir.dt.float16
F32 = mybir.dt.float32
F8 = mybir.dt.float8e4
E4 = ml_dtypes.float8_e4m3
AF = mybir.ActivationFunctionType
DR = mybir.MatmulPerfMode.DoubleRow

last_result = None  # BassKernelResults of the most recent run (for profiling)
_prog_cache = {}


def _foldT(M):
    """M [X cols, Rd rows] -> tile [128, (Rd/128)*X]; tile[p, q*X+x] = M[x, q*128+p]."""
    X, Rd = M.shape
    q = Rd // 128
    return np.ascontiguousarray(M.reshape(X, q, 128).transpose(2, 1, 0).reshape(128, q * X))


def _ro_rows(W):
    """Reorder gate rows [i,f,g,o] -> [i,f,o,2g] (W [4H, K])."""
    i, f, g, o = np.split(W, 4, axis=0)
    return np.concatenate([i, f, o, 2.0 * g], axis=0)


def _ro_cols(A):
    """Reorder gate cols [i,f,g,o] -> [i,f,o,2g] (A [..., 4H])."""
    i, f, g, o = np.split(A, 4, axis=-1)
    return np.concatenate([i, f, o, 2.0 * g], axis=-1)


def _dr8(W):
    """W [4H(reordered), K] -> [K//256, 128, 2, 4H] e4m3 DoubleRow lhsT chunks."""
    G, K = W.shape
    J = K // 256
    arr = W.T.reshape(J, 2, 128, G).transpose(0, 2, 1, 3)
    return np.ascontiguousarray(arr).astype(E4)


def _wT(W, in_dim):
    """W [G, in_dim] -> [in_dim//128, 128, G] chunks of W.T (fp16 matmul lhsT)."""
    G = W.shape[0]
    return np.ascontiguousarray(W.reshape(G, in_dim // 128, 128).transpose(1, 2, 0))


def _build_program(use_ctx_bias):
    nc = bacc.Bacc(None, target_bir_lowering=False)

    # ---- DRAM I/O ----
    d_w1h8 = nc.dram_tensor("w1h8", [3, 128, 2, 1024], F8, kind="ExternalInput")
    d_wci8 = nc.dram_tensor("wci8", [3, 128, 2, 1024], F8, kind="ExternalInput")
    d_wch8 = nc.dram_tensor("wch8", [128, 2, 1024], F8, kind="ExternalInput")
    d_w1n8 = nc.dram_tensor("w1n8", [3, 128, 2, 1024], F8, kind="ExternalInput")
    d_wo = nc.dram_tensor("wo", [3, 2, 128, 130], F16, kind="ExternalInput")
    d_xc1 = nc.dram_tensor("xc1", [3, 128, 2048], F16, kind="ExternalInput")
    d_hinit16 = nc.dram_tensor("hinit16", [128, 512], F16, kind="ExternalInput")
    d_hinit8 = nc.dram_tensor("hinit8", [128, 2, 256], F8, kind="ExternalInput")
    d_xa0 = nc.dram_tensor("xa0", [3, 128, 2048], F16, kind="ExternalInput")
    d_xb = nc.dram_tensor("xb", [S, 3, 128, 2048], F16, kind="ExternalInput")
    d_boutA = nc.dram_tensor("boutA", [3, 128, 1], F32, kind="ExternalInput")
    d_boutB = nc.dram_tensor("boutB", [3, 2, 1], F32, kind="ExternalInput")
    if use_ctx_bias:
        d_bcb = nc.dram_tensor("bcb", [128, 2048], F16, kind="ExternalInput")
    d_out = nc.dram_tensor("out", [S, 3, 130, R], F16, kind="ExternalOutput")

    from contextlib import ExitStack
    with tile.TileContext(nc) as tc, ExitStack() as es:
        const = es.enter_context(tc.tile_pool(name="const", bufs=1))
        psum = es.enter_context(tc.tile_pool(name="psum", bufs=2, space=MemorySpace.PSUM))
        sgp = es.enter_context(tc.tile_pool(name="sgp", bufs=6))
        tmp = es.enter_context(tc.tile_pool(name="tmp", bufs=4))
        tcp = es.enter_context(tc.tile_pool(name="tcp", bufs=3))
        npool = es.enter_context(tc.tile_pool(name="npool", bufs=3))
        hpool = es.enter_context(tc.tile_pool(name="hpool", bufs=2))
        hcpool = es.enter_context(tc.tile_pool(name="hcpool", bufs=4))
        cpool = es.enter_context(tc.tile_pool(name="cpool", bufs=2))
        stg = es.enter_context(tc.tile_pool(name="stg", bufs=3))

        def cload(name, dram_ap, shape, dtype):
            t = const.tile(shape, dtype, tag=name)
            nc.sync.dma_start(t[:], dram_ap)
            return t

        # order: first-needed consts first so PE starts before preload ends
        hinit8 = cload("hinit8", d_hinit8[:], [128, 2, 256], F8)
        hinit16 = cload("hinit16", d_hinit16[:], [128, 512], F16)
        xa0 = [cload(f"xa0_{i}", d_xa0[i], [128, 2048], F16) for i in range(3)]
        w1h8 = [cload(f"w1h8_{i}", d_w1h8[i], [128, 2, 1024], F8) for i in range(3)]
        wci8 = [cload(f"wci8_{j}", d_wci8[j], [128, 2, 1024], F8) for j in range(3)]
        wch8 = cload("wch8", d_wch8[:], [128, 2, 1024], F8)
        w1n8 = [cload(f"w1n8_{i}", d_w1n8[i], [128, 2, 1024], F8) for i in range(3)]
        xc1 = [cload(f"xc1_{i}", d_xc1[i], [128, 2048], F16) for i in range(3)]
        wo = [[cload(f"wo_{i}_{k}", d_wo[i, k], [128, 130], F16) for k in range(2)]
              for i in range(3)]
        boutA = [cload(f"boutA_{i}", d_boutA[i], [128, 1], F32) for i in range(3)]
        boutB = [cload(f"boutB_{i}", d_boutB[i], [2, 1], F32) for i in range(3)]
        bcb = cload("bcb", d_bcb[:], [128, 2048], F16) if use_ctx_bias else None

        ident = const.tile([128, 128], F16, tag="ident")
        make_identity(nc, ident[:])
        zc3 = const.tile([128, 1536], F16, tag="zc3")
        nc.gpsimd.memset(zc3[:], 0.0)
        zc1 = const.tile([128, 512], F16, tag="zc1")
        nc.gpsimd.memset(zc1[:], 0.0)

        def h8v(t):
            return t[:].rearrange("p i n -> p (i n)")

        def gates_mm(dr_pairs, xadd, tag):
            """Accumulate gate tile [128,2048] = ident@xadd + sum of DoubleRow
            pairs. dr_pairs: list of (w8_tile, rhs8_tile). Returns psum tile."""
            pt = psum.tile([128, 2048], F32, tag="g", name=tag)
            if xadd is not None:
                for gi in range(4):
                    nc.tensor.matmul(pt[:, gi * 512:(gi + 1) * 512], ident[:],
                                     xadd[:, gi * 512:(gi + 1) * 512],
                                     start=True, stop=False, skip_group_check=True)
            for m in range(8):
                outap = pt[:, m * 256:(m + 1) * 256]
                for j, (w8, rhs8) in enumerate(dr_pairs):
                    nc.tensor.matmul(outap, w8[:, :, m * 128:(m + 1) * 128], rhs8[:],
                                     start=(xadd is None and j == 0),
                                     stop=(j == len(dr_pairs) - 1),
                                     perf_mode=DR, skip_group_check=True)
            return pt

        def cell_front(pt, c_prev, c_out_ap, spine=False):
            """sigmoid + c update. c_prev: fp16 [128,512] AP; c_out_ap: fp16
            [128,512] AP (may be a slice of a group tile). Returns sg tile."""
            sg = sgp.tile([128, 2048], F16, tag="sg")
            nc.scalar.activation(sg[:], pt[:], AF.Sigmoid)
            u = tmp.tile([128, 512], F16, tag="u")
            nc.vector.tensor_scalar(u[:], sg[:, 1536:2048], 2.0, -1.0,
                                    mybir.AluOpType.mult, mybir.AluOpType.add)
            m1 = tmp.tile([128, 512], F16, tag="m1")
            nc.vector.tensor_mul(m1[:], u[:], sg[:, 0:512])
            cf = tmp.tile([128, 512], F16, tag="cf")
            if spine:
                nc.vector.tensor_mul(cf[:], sg[:, 512:1024], c_prev)
            else:
                nc.gpsimd.tensor_mul(cf[:], sg[:, 512:1024], c_prev)
            nc.vector.tensor_add(c_out_ap, cf[:], m1[:])
            return sg

        def h_out(sg, tc_ap, tag8, tag16=None, pool=None):
            """h = sigmoid(o) * tanh(c). Returns (h8 tile, h16 tile|None)."""
            h8 = (pool or hpool).tile([128, 2, 256], F8, tag=tag8)
            nc.vector.tensor_mul(h8v(h8), sg[:, 1024:1536], tc_ap)
            h16 = None
            if tag16 is not None:
                h16 = hpool.tile([128, 512], F16, tag=tag16)
                nc.vector.tensor_mul(h16[:], sg[:, 1024:1536], tc_ap)
            return h8, h16

        # ---- state ----
        h1_8 = [hinit8, hinit8, hinit8]     # lstm1 h (after "again")
        h2_8 = [hinit8, hinit8, hinit8]
        h2_16 = [hinit16, hinit16, hinit16]
        hc_8 = hinit8
        cg1 = zc3    # c1 states, grouped [128, 3*512] (slice i = stream i)
        cg2 = zc3
        cc = zc1

        def out_proj(i, h1v16_i, h216_i, s):
            hsum = stg.tile([128, 512], F16, tag="hsum")
            nc.vector.tensor_add(hsum[:], h1v16_i[:], h216_i[:])
            tout = psum.tile([128, 2048], F32, tag="g", name=f"tout_{s}_{i}")
            for k in range(2):
                nc.tensor.matmul(tout[:, 0:R], wo[i][k][:, 0:128],
                                 hsum[:, k * R:(k + 1) * R],
                                 start=(k == 0), stop=(k == 1),
                                 skip_group_check=True)
            for k in range(2):
                nc.tensor.matmul(tout[0:2, R:2 * R], wo[i][k][:, 128:130],
                                 hsum[:, k * R:(k + 1) * R],
                                 start=(k == 0), stop=(k == 1),
                                 skip_group_check=True)
            stage = stg.tile([128, 512], F16, tag="stage")
            nc.vector.tensor_scalar_add(stage[:, 0:R], tout[:, 0:R], boutA[i][:])
            nc.vector.tensor_scalar_add(stage[0:2, R:2 * R], tout[0:2, R:2 * R],
                                        boutB[i][:])
            nc.sync.dma_start(d_out[s, i, 0:128, :], stage[:, 0:R])
            nc.sync.dma_start(d_out[s, i, 128:130, :], stage[0:2, R:2 * R])

        xb_prev = None
        pending_outs = []
        for s in range(S):
            # stream this step's additive input term (teacher-forced note
            # contribution + xc1 + biases), prefetched by Tile
            xb_cur = []
            for i in range(3):
                t = npool.tile([128, 2048], F16, tag=f"xb_{i}")
                nc.sync.dma_start(t[:], d_xb[s, i])
                xb_cur.append(t)
            xa = xa0 if s == 0 else xb_prev

            # ---- vmap: lstm1 update for the 3 streams ----
            cg1_new = cpool.tile([128, 1536], F16, tag="cg1")
            sgv = []
            for i in range(3):
                pt = gates_mm([(w1h8[i], h1_8[i])], xa[i], f"gv_{s}_{i}")
                sgv.append(cell_front(pt, cg1[:, i * 512:(i + 1) * 512],
                                      cg1_new[:, i * 512:(i + 1) * 512]))
            cg1 = cg1_new
            tcv = tcp.tile([128, 1536], F16, tag="tcg")
            nc.scalar.activation(tcv[:], cg1[:], AF.Tanh)
            h1v_8, h1v_16 = [], []
            for i in range(3):
                a, b = h_out(sgv[i], tcv[:, i * 512:(i + 1) * 512],
                             f"h1v8_{i}", f"h1v16_{i}")
                h1v_8.append(a)
                h1v_16.append(b)

            # previous step's deferred output projections (PE fill work)
            for fn in pending_outs:
                fn()
            pending_outs = []

            # ---- ctx(0) gates + sigmoid (spine head) ----
            def ctx_cell(hin8, tag):
                pt = gates_mm([(wci8[j], hin8[j]) for j in range(3)]
                              + [(wch8, hc_8)], bcb, tag)
                cc_new = cpool.tile([128, 512], F16, tag="cc")
                sg = cell_front(pt, cc[:], cc_new[:], spine=True)
                tcc = tcp.tile([128, 512], F16, tag="tcc")
                nc.scalar.activation(tcc[:], cc_new[:], AF.Tanh)
                h8, _ = h_out(sg, tcc[:], "hc8", pool=hcpool)
                return h8, cc_new

            hc_8, cc = ctx_cell(h1v_8, f"gc_{s}_0")
            hcs = [hc_8]

            # ---- again: second lstm1 update (independent of ctx) ----
            cg1_new = cpool.tile([128, 1536], F16, tag="cg1")
            sga = []
            for i in range(3):
                pt = gates_mm([(w1h8[i], h1v_8[i])], xb_cur[i], f"ga_{s}_{i}")
                sga.append(cell_front(pt, cg1[:, i * 512:(i + 1) * 512],
                                      cg1_new[:, i * 512:(i + 1) * 512]))
            cg1 = cg1_new
            tca = tcp.tile([128, 1536], F16, tag="tcg")
            nc.scalar.activation(tca[:], cg1[:], AF.Tanh)
            h1_8 = []
            for i in range(3):
                a, _ = h_out(sga[i], tca[:, i * 512:(i + 1) * 512], f"h18_{i}")
                h1_8.append(a)

            # ---- ctx(1), ctx(2) (spine) ----
            hc_8, cc = ctx_cell([h1_8[0], h1v_8[1], h1v_8[2]], f"gc_{s}_1")
            hcs.append(hc_8)
            hc_8, cc = ctx_cell([h1_8[0], h1_8[1], h1v_8[2]], f"gc_{s}_2")
            hcs.append(hc_8)

            # ---- lstm2 for the 3 streams ----
            cg2_new = cpool.tile([128, 1536], F16, tag="cg2")
            sgl = []
            for i in range(3):
                pt = gates_mm([(w1n8[i], hcs[i]), (w1h8[i], h2_8[i])],
                              xc1[i], f"gl_{s}_{i}")
                sgl.append(cell_front(pt, cg2[:, i * 512:(i + 1) * 512],
                                      cg2_new[:, i * 512:(i + 1) * 512]))
            cg2 = cg2_new
            tcl = tcp.tile([128, 1536], F16, tag="tcg")
            nc.scalar.activation(tcl[:], cg2[:], AF.Tanh)
            h2_8, h2_16 = [], []
            for i in range(3):
                a, b = h_out(sgl[i], tcl[:, i * 512:(i + 1) * 512],
                             f"h28_{i}", f"h216_{i}")
                h2_8.append(a)
                h2_16.append(b)

            pending_outs = [
                (lambda i=i, a=h1v_16[i], b=h2_16[i], s_=s: out_proj(i, a, b, s_))
                for i in range(3)]
            xb_prev = xb_cur
        for fn in pending_outs:
            fn()

    nc.compile()
    return nc


def kernel(c, target, length, W_hid, b_hid, W1_ih, W1_hh, b1_ih, b1_hh,
           Wc_ih, Wc_hh, bc_ih, bc_hh, emb, Wout, bout):
    global last_result
    c = np.asarray(c, np.float32)
    tgt = np.asarray(target).astype(np.int64)
    W_hid = np.asarray(W_hid, np.float32)
    b_hid = np.asarray(b_hid, np.float32)
    W1_ih = np.asarray(W1_ih, np.float32)
    W1_hh = np.asarray(W1_hh, np.float32)
    b1 = np.asarray(b1_ih, np.float32) + np.asarray(b1_hh, np.float32)
    Wc_ih = np.asarray(Wc_ih, np.float32)
    Wc_hh = np.asarray(Wc_hh, np.float32)
    bc = np.asarray(bc_ih, np.float32) + np.asarray(bc_hh, np.float32)
    emb = np.asarray(emb, np.float32)
    Wout = np.asarray(Wout, np.float32)
    bout = np.asarray(bout, np.float32)
    L = int(length)
    assert L == NB * S and c.shape == (B, NB + 1, Dd)

    f16 = np.float16
    use_ctx_bias = bool(np.any(bc != 0.0))

    # ---- replicated weight prep (gate rows reordered [i,f,o,2g]) ----
    w1h8 = np.stack([_dr8(_ro_rows(W1_hh[i]))[0] for i in range(3)])
    wci8 = _dr8(_ro_rows(Wc_ih))                       # [3,128,2,1024]
    wch8 = _dr8(_ro_rows(Wc_hh))[0]
    w1n8 = np.stack([_dr8(_ro_rows(W1_ih[i][:, :Dd]))[0] for i in range(3)])
    wo = np.stack([_wT(Wout[i], Hh) for i in range(3)]).astype(f16)
    boutA = np.ascontiguousarray(bout[:, :128, None])
    boutB = np.ascontiguousarray(bout[:, 128:130, None])
    bcb = _foldT(np.broadcast_to(_ro_cols(bc)[None, :], (R, 4 * Hh))).astype(f16)

    # full-batch fp32 precomputes
    h_init_full = np.tanh(np.einsum('bnd,hd->bnh', c[:, :NB], W_hid[:Hh]) + b_hid[:Hh])
    # note contribution per vocab entry: NEt[i] = emb[i] @ W1n[i].T  [V, 4H]
    NEt = np.stack([emb[i] @ W1_ih[i][:, :Dd].T for i in range(3)])
    in_maps = []
    for r in range(NCORES):
        cs = c[r * BL:(r + 1) * BL]           # [BL, 17, D]
        CT = cs[:, 1:NB + 1].transpose(1, 0, 2).reshape(R, Dd)      # x=(bar,bl)
        HI = h_init_full[r * BL:(r + 1) * BL].transpose(1, 0, 2).reshape(R, Hh)
        xc1f = [CT @ W1_ih[i][:, Dd:].T + b1[i] for i in range(3)]  # [R, 4H]
        xc1 = np.stack([_foldT(_ro_cols(x)) for x in xc1f]).astype(f16)
        hinit16 = _foldT(HI).astype(f16)
        hinit8 = hinit16.astype(E4).reshape(128, 2, 256)
        tg = tgt[:, r * BL:(r + 1) * BL]      # [3, BL, 256]
        # notes entering step 0: bar0 -> token 0; else target at bar*16-1
        tokA0 = np.empty((3, R), np.int64)
        for i in range(3):
            tokA0[i] = np.concatenate(
                [np.zeros(BL, np.int64)] +
                [tg[i, :, bar * S - 1] for bar in range(1, NB)])
        xa0 = np.stack([_foldT(_ro_cols(NEt[i][tokA0[i]] + xc1f[i]))
                        for i in range(3)]).astype(f16)
        # combined additive input term at step s (teacher forcing)
        tr = tg.reshape(3, BL, NB, S)         # [i, bl, bar, s]
        xbarr = np.empty((S, 3, 128, 2048), f16)
        for s in range(S):
            for i in range(3):
                toks = tr[i, :, :, s].T.reshape(R)   # x = bar*BL+bl
                xbarr[s, i] = _foldT(_ro_cols(NEt[i][toks] + xc1f[i])).astype(f16)
        m = dict(w1h8=w1h8, wci8=wci8, wch8=wch8, w1n8=w1n8, wo=wo, xc1=xc1,
                 hinit16=hinit16, hinit8=hinit8, xa0=xa0, xb=xbarr,
                 boutA=boutA, boutB=boutB)
        if use_ctx_bias:
            m["bcb"] = bcb
        in_maps.append(m)

    key = use_ctx_bias
    if key not in _prog_cache:
        _prog_cache[key] = _build_program(use_ctx_bias)
    nc = _prog_cache[key]

    last_result = run_bass_kernel_spmd(nc, in_maps, core_ids=list(range(NCORES)))

    out_full = np.empty((3, B, L, Vv), np.float32)
    for r in range(NCORES):
        A = last_result.results[r]["out"].astype(np.float32)   # [S, 3, 130, R]
        A = A.reshape(S, 3, Vv, NB, BL).transpose(1, 4, 3, 0, 2)  # [3, bl, bar, s, V]
        out_full[:, r * BL:(r + 1) * BL] = A.reshape(3, BL, L, Vv)
    return out_full
